# revision 25
# baseline (speedup 1.0000x reference)
"""Trainium2 Bass kernel for nn_LoopedMLP (moe_routing).

Reference semantics (B=8, T=1024, C=1024, ITER=4096, FULL=12288, R=0.7):
a 3-step scan; each step computes
    y = relu((x @ Wm^T) * active_mask) @ Wp^T
then a control net on mean-pooled y picks a new top-4096 column mask, and a
batch-mean "novelty" below R freezes the state for remaining steps.

Because relu((x@Wm^T)*mask) @ Wp^T only touches the masked 4096 columns, each
step is exactly a dense per-sample MLP over the *gathered* active columns:
    y = relu(x @ A^T) @ B,   A = Wm[idx], B = Wp[:, idx]^T,  |idx| = 4096.

Strategy: data-parallel over B (1 sample per NeuronCore, 8 cores). The device
kernel computes the gathered MLP; all routing logic (control net, top-k,
novelty, done) runs on host between launches. With random weights the step-1
novelty is ~2/3 < R, so in practice exactly one device launch happens.

Device compute per core: two back-to-back matmul chains in fp16 (stationary
weights via pipelined FWL ldweights, 1 moving column/cycle at N=512) with
fp32 PSUM accumulation:
  phase 1: H^T[j,t]   = relu( sum_c A^T[c,j] * X^T[c,t] )  (j=4096, t=1024, c=1024)
  phase 2: Y^T[c,t]   = sum_j B[j,c] * H^T[j,t]
Measured on TRN2 (NTFF): ~243-250 us/launch vs a 218 us PE roofline (1024
matmuls of [128x128]@[128x512] issuing back-to-back at 216 ns, zero
steady-state stalls; dummy warmup matmuls pre-warm the HAM clock gate
during the input-DMA wait); final output rel err vs the fp32 reference
~4e-4. Occasional chip-level P0 power throttling can stretch a run to
~290 us (PE at ~2.0 GHz) — outside kernel control.
"""

import os
import sys

import numpy as np


def _ensure_concourse():
    try:
        import concourse  # noqa: F401
    except ImportError:
        for p in ("/opt/trn_rl_repo", "/root/.axon_site/_ro/trn_rl_repo"):
            if os.path.isdir(p) and p not in sys.path:
                sys.path.insert(0, p)
        import concourse  # noqa: F401


N_EMBD = 1024
T_SEQ = 1024
ITER = 4096
FULL = 12288
R_NOVELTY = 0.7
NCORES = 8
JT = ITER // 128   # 32 j-tiles
CT = N_EMBD // 128  # 8 c-tiles

_STATE: dict = {}


# ---------------------------------------------------------------- device side

def _build_nc(n_repeat=1):
    _ensure_concourse()
    import concourse.tile as tile
    from concourse import bacc, mybir
    from concourse.bass import ts

    f32 = mybir.dt.float32
    f32r = mybir.dt.float32r
    relu = mybir.ActivationFunctionType.Relu

    f16 = mybir.dt.float16
    nc = bacc.Bacc("TRN2", target_bir_lowering=False, debug=False,
                   num_devices=NCORES)
    xa = nc.dram_tensor("xt", [2, CT, 128, T_SEQ // 2], f16,
                        kind="ExternalInput").ap()
    aa = nc.dram_tensor("at", [JT, 128, N_EMBD], f16, kind="ExternalInput").ap()
    ba = nc.dram_tensor("bt", [JT, 128, N_EMBD], f16, kind="ExternalInput").ap()
    # output is Y^T tiles: [t-half, c-tile, 128 c, 512 t]
    ya = nc.dram_tensor("y", [2, 8, 128, 512], f32, kind="ExternalOutput").ap()

    with tile.TileContext(nc) as tc:
        with (
            tc.tile_pool(name="xt", bufs=2 * CT) as xt_pool,
            tc.tile_pool(name="ht", bufs=JT) as ht_pool,
            tc.tile_pool(name="at", bufs=6) as at_pool,
            tc.tile_pool(name="bt", bufs=JT) as bt_pool,
            tc.tile_pool(name="yo", bufs=4) as yo_pool,
            tc.tile_pool(name="ps", bufs=8, space="PSUM") as ps_pool,
        ):
          for rep in range(n_repeat):
            # PE warmup: dummy matmuls on a zeroed tile run during the
            # input-DMA wait, so the HAM clock-gate reaches 8/8 before the
            # real stream starts (saves the ~3.4us cold ramp at 1.2 GHz).
            wt = xt_pool.tile([128, 512], f16, tag="warm", name=f"w{rep}")
            nc.vector.memset(wt[:], 0)
            wps = ps_pool.tile([128, 512], f32, tag="ps", name=f"wps{rep}")
            for i in range(18):
                nc.tensor.matmul(wps[:], lhsT=wt[:, ts(0, 128)], rhs=wt[:],
                                 start=True, stop=True)

            # issue order matches the head's consumption order (ct-major)
            xts = [[None] * 2 for _ in range(CT)]
            for ct in range(CT):
                for th in range(2):
                    t = xt_pool.tile([128, T_SEQ // 2], f16, tag="xt",
                                     name=f"x{rep}_{th}_{ct}")
                    eng = nc.gpsimd if th == 0 else nc.scalar
                    eng.dma_start(out=t[:], in_=xa[th, ct])
                    xts[ct][th] = t

            # phase 1: H^T, one [128 j, 1024 t] tile per j-tile
            # stationary = A^T slice (fp16 -> pipelined FWL ldweights),
            # moving = X^T half (fp16), fp32 PSUM accumulation.
            # The first HEAD_JT j-tiles run ct-outer so matmuls start as
            # soon as the first xt half-tiles arrive (ramp overlap).
            HEAD_JT = 3
            hts = []
            head_att, head_ps = [], []
            for jt in range(HEAD_JT):
                att = at_pool.tile([128, N_EMBD], f16, tag="at",
                                   name=f"a{rep}_{jt}")
                nc.sync.dma_start(out=att[:], in_=aa[jt])
                head_att.append(att)
                head_ps.append([ps_pool.tile([128, 512], f32, tag="ps",
                                             name=f"hps{rep}_{jt}_{i}")
                                for i in range(2)])
            for ct in range(CT):
                for th in range(2):
                    rhs = xts[ct][th][:]
                    for jt in range(HEAD_JT):
                        nc.tensor.matmul(
                            head_ps[jt][th][:],
                            lhsT=head_att[jt][:, ts(ct, 128)],
                            rhs=rhs, start=(ct == 0), stop=(ct == CT - 1))
            for jt in range(HEAD_JT):
                htt = ht_pool.tile([128, T_SEQ], f16, tag="ht",
                                   name=f"h{rep}_{jt}")
                for th in range(2):
                    nc.scalar.activation(htt[:, ts(th, 512)],
                                         head_ps[jt][th][:], relu)
                hts.append(htt)

            for jt in range(HEAD_JT, JT):
                att = at_pool.tile([128, N_EMBD], f16, tag="at",
                                   name=f"a{rep}_{jt}")
                nc.sync.dma_start(out=att[:], in_=aa[jt])
                ps = [ps_pool.tile([128, 512], f32, tag="ps", name=f"hps{rep}_{jt}_{i}") for i in range(2)]
                for ct in range(CT):
                    lhs = att[:, ts(ct, 128)]
                    for th in range(2):
                        nc.tensor.matmul(
                            ps[th][:], lhsT=lhs,
                            rhs=xts[ct][th][:],
                            start=(ct == 0), stop=(ct == CT - 1))
                htt = ht_pool.tile([128, T_SEQ], f16, tag="ht",
                                   name=f"h{rep}_{jt}")
                for th in range(2):
                    nc.scalar.activation(htt[:, ts(th, 512)], ps[th][:], relu)
                hts.append(htt)

            # B tiles prefetched once, kept resident (fp16: 64KB/partition);
            # emitted after phase 1 so the A stream wins DMA priority.
            bts = []
            for jt in range(JT):
                btt = bt_pool.tile([128, N_EMBD], f16, tag="bt",
                                   name=f"b{rep}_{jt}")
                nc.gpsimd.dma_start(out=btt[:], in_=ba[jt])
                bts.append(btt)

            # phase 2: Y^T[c,t] accumulated over j. c8-outer/jt-inner:
            # each accumulation group finishes 32 MMs before the next, so
            # all copies/output DMAs except the last hide under the MM
            # stream (and only ~2 PSUM banks are ever live).
            for th in range(2):
                for c8 in range(8):
                    pst = ps_pool.tile([128, 512], f32, tag="ps",
                                       name=f"yps{rep}_{th}_{c8}")
                    for jt in range(JT):
                        nc.tensor.matmul(
                            pst[:], lhsT=bts[jt][:, ts(c8, 128)],
                            rhs=hts[jt][:, ts(th, 512)],
                            start=(jt == 0), stop=(jt == JT - 1))
                    yo = yo_pool.tile([128, 512], f32, tag="yo",
                                      name=f"y{rep}_{th}_{c8}")
                    if c8 % 2 == 0:
                        nc.vector.tensor_copy(yo[:], pst[:])
                        nc.sync.dma_start(out=ya[th, c8], in_=yo[:])
                    else:
                        nc.scalar.copy(yo[:], pst[:])
                        nc.scalar.dma_start(out=ya[th, c8], in_=yo[:])

    nc.compile()
    return nc


class _Runner:
    """Persistent jitted SPMD dispatcher (mirrors bass2jax.run_bass_via_pjrt's
    multi-core branch, but reuses one jax.jit across calls)."""

    def __init__(self, nc):
        _ensure_concourse()
        import jax
        import concourse.mybir as mybir
        from concourse import bass2jax
        from jax.experimental.shard_map import shard_map
        from jax.sharding import Mesh, PartitionSpec

        bass2jax.install_neuronx_cc_hook()
        self.nc = nc
        partition_name = (nc.partition_id_tensor.name
                          if nc.partition_id_tensor else None)
        in_names, out_names, out_avals, zero_shapes = [], [], [], []
        for alloc in nc.m.functions[0].allocations:
            if not isinstance(alloc, mybir.MemoryLocationSet):
                continue
            name = alloc.memorylocations[0].name
            if alloc.kind == "ExternalInput":
                if name != partition_name:
                    in_names.append(name)
            elif alloc.kind == "ExternalOutput":
                shape = tuple(alloc.tensor_shape)
                dtype = mybir.dt.np(alloc.dtype)
                out_names.append(name)
                out_avals.append(jax.core.ShapedArray(shape, dtype))
                zero_shapes.append((shape, dtype))
        self.in_names = list(in_names)
        self.out_names = out_names
        self.out_avals = out_avals
        self.zero_shapes = zero_shapes
        n_params = len(in_names)
        all_in_names = in_names + out_names
        if partition_name is not None:
            all_in_names.append(partition_name)

        def _body(*args):
            operands = list(args)
            if partition_name is not None:
                operands.append(bass2jax.partition_id_tensor())
            outs = bass2jax._bass_exec_p.bind(
                *operands,
                out_avals=tuple(out_avals),
                in_names=tuple(all_in_names),
                out_names=tuple(out_names),
                lowering_input_output_aliases=(),
                sim_require_finite=True,
                sim_require_nnan=True,
                nc=nc,
            )
            return tuple(outs)

        devices = jax.devices()[:NCORES]
        assert len(devices) == NCORES
        self.mesh = Mesh(np.asarray(devices), ("core",))
        n_outs = len(out_names)
        in_specs = (PartitionSpec("core"),) * (n_params + n_outs)
        out_specs = (PartitionSpec("core"),) * n_outs
        self.donate = tuple(range(n_params, n_params + n_outs))
        self.fn = jax.jit(
            shard_map(_body, mesh=self.mesh, in_specs=in_specs,
                      out_specs=out_specs, check_rep=False),
            donate_argnums=self.donate, keep_unused=True)

    def concat_inputs(self, in_maps):
        return [np.concatenate([np.asarray(m[n]) for m in in_maps], axis=0)
                for n in self.in_names]

    def zero_outs(self):
        return [np.zeros((NCORES * s[0], *s[1:]), d)
                for (s, d) in self.zero_shapes]

    def __call__(self, in_maps):
        concat_in = self.concat_inputs(in_maps)
        out_arrs = self.fn(*concat_in, *self.zero_outs())
        return [
            {n: np.asarray(out_arrs[i]).reshape(NCORES, *self.out_avals[i].shape)[c]
             for i, n in enumerate(self.out_names)}
            for c in range(NCORES)
        ]


def _get_runner():
    if "runner" not in _STATE:
        nc = _build_nc()
        _STATE["nc"] = nc
        _STATE["runner"] = _Runner(nc)
    return _STATE["runner"]


# ------------------------------------------------------------------ host side

def _tile_A(A):
    """(4096, 1024) row-gathered Wm -> fp16 dram 'at' layout [jt, ci, ct*128+jj]."""
    return np.ascontiguousarray(
        A.reshape(JT, 128, CT, 128).transpose(0, 3, 2, 1)).reshape(
            JT, 128, N_EMBD).astype(np.float16)


def _tile_B(Bm):
    """(4096, 1024) row-gathered Wp^T -> fp16 dram 'bt' layout [jt, jj, c]."""
    return np.ascontiguousarray(Bm).reshape(JT, 128, N_EMBD).astype(np.float16)


def _tile_X(xc):
    """(B, 1024 t, 1024 c) -> per-core fp16 dram 'xt' layout [th, ct, ci, tt]."""
    b = xc.shape[0]
    arr = np.ascontiguousarray(xc.transpose(0, 2, 1)).reshape(
        b, CT, 128, 2, T_SEQ // 2).astype(np.float16)
    return np.ascontiguousarray(arr.transpose(0, 3, 1, 2, 4))


def _untile_Y(y_tiled):
    """dram 'y' [th, c8, ci, tt] (Y^T tiles) -> f32 (1024 t, 1024 c)."""
    return np.ascontiguousarray(
        y_tiled.transpose(0, 3, 1, 2)).reshape(T_SEQ, N_EMBD).astype(
            np.float32, copy=False)


def _device_forward(xc, A_list, B_list):
    """y[b] = relu(xc[b] @ A_list[b]^T) @ B_list[b] for 8 cores at once."""
    xts = _tile_X(xc)
    in_maps = []
    for b in range(NCORES):
        in_maps.append({"xt": xts[b], "at": A_list[b], "bt": B_list[b]})
    try:
        results = _get_runner()(in_maps)
    except Exception:
        # fall back to the supported dispatch path (fresh jit per call)
        from concourse.bass_utils import run_bass_kernel_spmd
        if "nc" not in _STATE:
            _STATE["nc"] = _build_nc()
        results = run_bass_kernel_spmd(
            _STATE["nc"], in_maps, list(range(NCORES))).results
    return np.stack([_untile_Y(results[b]["y"]) for b in range(NCORES)])


def _topk_mask(ck, k):
    # matches jax.lax.top_k tie-breaking (first index wins) via stable argsort
    order = np.argsort(-ck, axis=1, kind="stable")[:, :k]
    mask = np.zeros_like(ck)
    np.put_along_axis(mask, order, 1.0, axis=1)
    return mask


def kernel(x, Wm, Wp, Wc1, Wc2):
    x = np.ascontiguousarray(np.asarray(x, dtype=np.float32))
    Wm = np.ascontiguousarray(np.asarray(Wm, dtype=np.float32))
    Wp = np.ascontiguousarray(np.asarray(Wp, dtype=np.float32))
    Wc1 = np.asarray(Wc1, dtype=np.float32)
    Wc2 = np.asarray(Wc2, dtype=np.float32)
    B = x.shape[0]
    assert B == NCORES and x.shape[1] == T_SEQ and x.shape[2] == N_EMBD

    WpT = None  # lazily built; only needed on non-base iterations
    base = np.zeros((B, FULL), np.float32)
    base[:, :ITER] = 1.0

    xc, active, history, done = x, base, base.copy(), False
    for _ in range(3):
        if done:
            break
        idxs = [np.flatnonzero(active[b]) for b in range(B)]
        is_base = all(ix.shape[0] == ITER and ix[0] == 0 and ix[-1] == ITER - 1
                      for ix in idxs) and all(
                          np.array_equal(ix, idxs[0]) for ix in idxs[1:])
        if is_base and np.array_equal(idxs[0], np.arange(ITER)):
            at = _tile_A(Wm[:ITER])
            bt = _tile_B(np.ascontiguousarray(Wp[:, :ITER].T))
            A_list = [at] * B
            B_list = [bt] * B
        else:
            if WpT is None:
                WpT = np.ascontiguousarray(Wp.T)
            A_list = [_tile_A(np.ascontiguousarray(Wm[ix])) for ix in idxs]
            B_list = [_tile_B(WpT[ix]) for ix in idxs]

        y = _device_forward(xc, A_list, B_list)

        pooled = y.mean(axis=1)
        ck = np.maximum(pooled @ Wc1.T, 0.0) @ Wc2.T
        new_mask = _topk_mask(ck, ITER)
        combined = np.clip(history + new_mask, 0.0, 1.0)
        novelty = (combined - history).sum(axis=1).mean() / ITER
        xc, active, history = y, new_mask, combined
        done = bool(novelty < R_NOVELTY)

    return xc.astype(np.float32, copy=False)


# revision 27
# speedup vs baseline: 1.0119x; 1.0119x over previous
"""Trainium2 Bass kernel for nn_LoopedMLP (moe_routing).

Reference semantics (B=8, T=1024, C=1024, ITER=4096, FULL=12288, R=0.7):
a 3-step scan; each step computes
    y = relu((x @ Wm^T) * active_mask) @ Wp^T
then a control net on mean-pooled y picks a new top-4096 column mask, and a
batch-mean "novelty" below R freezes the state for remaining steps.

Because relu((x@Wm^T)*mask) @ Wp^T only touches the masked 4096 columns, each
step is exactly a dense per-sample MLP over the *gathered* active columns:
    y = relu(x @ A^T) @ B,   A = Wm[idx], B = Wp[:, idx]^T,  |idx| = 4096.

Strategy: data-parallel over B (1 sample per NeuronCore, 8 cores). The device
kernel computes the gathered MLP; all routing logic (control net, top-k,
novelty, done) runs on host between launches. With random weights the step-1
novelty is ~2/3 < R, so in practice exactly one device launch happens.

Device compute per core: two back-to-back matmul chains in fp16 (stationary
weights via pipelined FWL ldweights, 1 moving column/cycle at N=512) with
fp32 PSUM accumulation:
  phase 1: H^T[j,t]   = relu( sum_c A^T[c,j] * X^T[c,t] )  (j=4096, t=1024, c=1024)
  phase 2: Y^T[c,t]   = sum_j B[j,c] * H^T[j,t]
Measured on TRN2 (NTFF): ~242-245 us/launch vs a 218 us PE roofline (1024
matmuls of [128x128]@[128x512] issuing back-to-back at 216 ns, zero
steady-state stalls; dummy warmup matmuls pre-warm the HAM clock gate
during the input-DMA wait); final output rel err vs the fp32 reference
~4e-4. Occasional chip-level P0 power throttling can stretch a run to
~290 us (PE at ~2.0 GHz) — outside kernel control.
"""

import os
import sys

import numpy as np


def _ensure_concourse():
    try:
        import concourse  # noqa: F401
    except ImportError:
        for p in ("/opt/trn_rl_repo", "/root/.axon_site/_ro/trn_rl_repo"):
            if os.path.isdir(p) and p not in sys.path:
                sys.path.insert(0, p)
        import concourse  # noqa: F401


N_EMBD = 1024
T_SEQ = 1024
ITER = 4096
FULL = 12288
R_NOVELTY = 0.7
NCORES = 8
JT = ITER // 128   # 32 j-tiles
CT = N_EMBD // 128  # 8 c-tiles

_STATE: dict = {}


# ---------------------------------------------------------------- device side

def _build_nc(n_repeat=1):
    _ensure_concourse()
    import concourse.tile as tile
    from concourse import bacc, mybir
    from concourse.bass import ts

    f32 = mybir.dt.float32
    f32r = mybir.dt.float32r
    relu = mybir.ActivationFunctionType.Relu

    f16 = mybir.dt.float16
    nc = bacc.Bacc("TRN2", target_bir_lowering=False, debug=False,
                   num_devices=NCORES)
    xa = nc.dram_tensor("xt", [CT, 128, T_SEQ], f16,
                        kind="ExternalInput").ap()
    aa = nc.dram_tensor("at", [JT, 128, N_EMBD], f16, kind="ExternalInput").ap()
    ba = nc.dram_tensor("bt", [JT, 128, N_EMBD], f16, kind="ExternalInput").ap()
    # output is Y^T tiles: [t-half, c-tile, 128 c, 512 t]
    ya = nc.dram_tensor("y", [2, 8, 128, 512], f32, kind="ExternalOutput").ap()

    with tile.TileContext(nc) as tc:
        with (
            tc.tile_pool(name="xt", bufs=CT + 1) as xt_pool,
            tc.tile_pool(name="ht", bufs=JT) as ht_pool,
            tc.tile_pool(name="at", bufs=6) as at_pool,
            tc.tile_pool(name="bt", bufs=JT) as bt_pool,
            tc.tile_pool(name="yo", bufs=4) as yo_pool,
            tc.tile_pool(name="ps", bufs=8, space="PSUM") as ps_pool,
        ):
          for rep in range(n_repeat):
            # PE warmup: dummy matmuls on a zeroed tile run during the
            # input-DMA wait, so the HAM clock-gate reaches 8/8 before the
            # real stream starts (saves the ~3.4us cold ramp at 1.2 GHz).
            wt = xt_pool.tile([128, 512], f16, tag="warm", name=f"w{rep}")
            nc.vector.memset(wt[:], 0)
            wps = ps_pool.tile([128, 512], f32, tag="ps", name=f"wps{rep}")
            for i in range(18):
                nc.tensor.matmul(wps[:], lhsT=wt[:, ts(0, 128)], rhs=wt[:],
                                 start=True, stop=True)

            # whole [128,1024] tiles: 2KB descriptor rows give ~80 GB/s
            # per DMA queue vs ~57 GB/s at 1KB; alternate the two queues
            # in ct (consumption) order.
            xts = []
            for ct in range(CT):
                t = xt_pool.tile([128, T_SEQ], f16, tag="xt",
                                 name=f"x{rep}_{ct}")
                eng = nc.gpsimd if ct % 2 == 0 else nc.scalar
                eng.dma_start(out=t[:], in_=xa[ct])
                xts.append(t)

            # phase 1: H^T, one [128 j, 1024 t] tile per j-tile
            # stationary = A^T slice (fp16 -> pipelined FWL ldweights),
            # moving = X^T half (fp16), fp32 PSUM accumulation.
            # The first HEAD_JT j-tiles run ct-outer so matmuls start as
            # soon as the first xt half-tiles arrive (ramp overlap).
            HEAD_JT = 2
            hts = []
            head_att, head_ps = [], []
            for jt in range(HEAD_JT):
                att = at_pool.tile([128, N_EMBD], f16, tag="at",
                                   name=f"a{rep}_{jt}")
                nc.sync.dma_start(out=att[:], in_=aa[jt])
                head_att.append(att)
                head_ps.append([ps_pool.tile([128, 512], f32, tag="ps",
                                             name=f"hps{rep}_{jt}_{i}")
                                for i in range(2)])
            for ct in range(CT):
                for th in range(2):
                    rhs = xts[ct][:, ts(th, 512)]
                    for jt in range(HEAD_JT):
                        nc.tensor.matmul(
                            head_ps[jt][th][:],
                            lhsT=head_att[jt][:, ts(ct, 128)],
                            rhs=rhs, start=(ct == 0), stop=(ct == CT - 1))
            for jt in range(HEAD_JT):
                htt = ht_pool.tile([128, T_SEQ], f16, tag="ht",
                                   name=f"h{rep}_{jt}")
                for th in range(2):
                    nc.scalar.activation(htt[:, ts(th, 512)],
                                         head_ps[jt][th][:], relu)
                hts.append(htt)

            for jt in range(HEAD_JT, JT):
                att = at_pool.tile([128, N_EMBD], f16, tag="at",
                                   name=f"a{rep}_{jt}")
                nc.sync.dma_start(out=att[:], in_=aa[jt])
                ps = [ps_pool.tile([128, 512], f32, tag="ps", name=f"hps{rep}_{jt}_{i}") for i in range(2)]
                for ct in range(CT):
                    lhs = att[:, ts(ct, 128)]
                    for th in range(2):
                        nc.tensor.matmul(
                            ps[th][:], lhsT=lhs,
                            rhs=xts[ct][:, ts(th, 512)],
                            start=(ct == 0), stop=(ct == CT - 1))
                htt = ht_pool.tile([128, T_SEQ], f16, tag="ht",
                                   name=f"h{rep}_{jt}")
                for th in range(2):
                    nc.scalar.activation(htt[:, ts(th, 512)], ps[th][:], relu)
                hts.append(htt)

            # B tiles prefetched once, kept resident (fp16: 64KB/partition);
            # emitted after phase 1 so the A stream wins DMA priority.
            bts = []
            for jt in range(JT):
                btt = bt_pool.tile([128, N_EMBD], f16, tag="bt",
                                   name=f"b{rep}_{jt}")
                nc.gpsimd.dma_start(out=btt[:], in_=ba[jt])
                bts.append(btt)

            # phase 2: Y^T[c,t] accumulated over j. c8-outer/jt-inner:
            # each accumulation group finishes 32 MMs before the next, so
            # all copies/output DMAs except the last hide under the MM
            # stream (and only ~2 PSUM banks are ever live).
            for th in range(2):
                for c8 in range(8):
                    pst = ps_pool.tile([128, 512], f32, tag="ps",
                                       name=f"yps{rep}_{th}_{c8}")
                    for jt in range(JT):
                        nc.tensor.matmul(
                            pst[:], lhsT=bts[jt][:, ts(c8, 128)],
                            rhs=hts[jt][:, ts(th, 512)],
                            start=(jt == 0), stop=(jt == JT - 1))
                    yo = yo_pool.tile([128, 512], f32, tag="yo",
                                      name=f"y{rep}_{th}_{c8}")
                    if c8 % 2 == 0:
                        nc.vector.tensor_copy(yo[:], pst[:])
                        nc.sync.dma_start(out=ya[th, c8], in_=yo[:])
                    else:
                        nc.scalar.copy(yo[:], pst[:])
                        nc.scalar.dma_start(out=ya[th, c8], in_=yo[:])

    nc.compile()
    return nc


class _Runner:
    """Persistent jitted SPMD dispatcher (mirrors bass2jax.run_bass_via_pjrt's
    multi-core branch, but reuses one jax.jit across calls)."""

    def __init__(self, nc):
        _ensure_concourse()
        import jax
        import concourse.mybir as mybir
        from concourse import bass2jax
        from jax.experimental.shard_map import shard_map
        from jax.sharding import Mesh, PartitionSpec

        bass2jax.install_neuronx_cc_hook()
        self.nc = nc
        partition_name = (nc.partition_id_tensor.name
                          if nc.partition_id_tensor else None)
        in_names, out_names, out_avals, zero_shapes = [], [], [], []
        for alloc in nc.m.functions[0].allocations:
            if not isinstance(alloc, mybir.MemoryLocationSet):
                continue
            name = alloc.memorylocations[0].name
            if alloc.kind == "ExternalInput":
                if name != partition_name:
                    in_names.append(name)
            elif alloc.kind == "ExternalOutput":
                shape = tuple(alloc.tensor_shape)
                dtype = mybir.dt.np(alloc.dtype)
                out_names.append(name)
                out_avals.append(jax.core.ShapedArray(shape, dtype))
                zero_shapes.append((shape, dtype))
        self.in_names = list(in_names)
        self.out_names = out_names
        self.out_avals = out_avals
        self.zero_shapes = zero_shapes
        n_params = len(in_names)
        all_in_names = in_names + out_names
        if partition_name is not None:
            all_in_names.append(partition_name)

        def _body(*args):
            operands = list(args)
            if partition_name is not None:
                operands.append(bass2jax.partition_id_tensor())
            outs = bass2jax._bass_exec_p.bind(
                *operands,
                out_avals=tuple(out_avals),
                in_names=tuple(all_in_names),
                out_names=tuple(out_names),
                lowering_input_output_aliases=(),
                sim_require_finite=True,
                sim_require_nnan=True,
                nc=nc,
            )
            return tuple(outs)

        devices = jax.devices()[:NCORES]
        assert len(devices) == NCORES
        self.mesh = Mesh(np.asarray(devices), ("core",))
        n_outs = len(out_names)
        in_specs = (PartitionSpec("core"),) * (n_params + n_outs)
        out_specs = (PartitionSpec("core"),) * n_outs
        self.donate = tuple(range(n_params, n_params + n_outs))
        self.fn = jax.jit(
            shard_map(_body, mesh=self.mesh, in_specs=in_specs,
                      out_specs=out_specs, check_rep=False),
            donate_argnums=self.donate, keep_unused=True)

    def concat_inputs(self, in_maps):
        return [np.concatenate([np.asarray(m[n]) for m in in_maps], axis=0)
                for n in self.in_names]

    def zero_outs(self):
        return [np.zeros((NCORES * s[0], *s[1:]), d)
                for (s, d) in self.zero_shapes]

    def __call__(self, in_maps):
        concat_in = self.concat_inputs(in_maps)
        out_arrs = self.fn(*concat_in, *self.zero_outs())
        return [
            {n: np.asarray(out_arrs[i]).reshape(NCORES, *self.out_avals[i].shape)[c]
             for i, n in enumerate(self.out_names)}
            for c in range(NCORES)
        ]


def _get_runner():
    if "runner" not in _STATE:
        nc = _build_nc()
        _STATE["nc"] = nc
        _STATE["runner"] = _Runner(nc)
    return _STATE["runner"]


# ------------------------------------------------------------------ host side

def _tile_A(A):
    """(4096, 1024) row-gathered Wm -> fp16 dram 'at' layout [jt, ci, ct*128+jj]."""
    return np.ascontiguousarray(
        A.reshape(JT, 128, CT, 128).transpose(0, 3, 2, 1)).reshape(
            JT, 128, N_EMBD).astype(np.float16)


def _tile_B(Bm):
    """(4096, 1024) row-gathered Wp^T -> fp16 dram 'bt' layout [jt, jj, c]."""
    return np.ascontiguousarray(Bm).reshape(JT, 128, N_EMBD).astype(np.float16)


def _tile_X(xc):
    """(B, 1024 t, 1024 c) -> per-core fp16 dram 'xt' layout [ct, ci, t]."""
    return np.ascontiguousarray(xc.transpose(0, 2, 1)).reshape(
        xc.shape[0], CT, 128, T_SEQ).astype(np.float16)


def _untile_Y(y_tiled):
    """dram 'y' [th, c8, ci, tt] (Y^T tiles) -> f32 (1024 t, 1024 c)."""
    return np.ascontiguousarray(
        y_tiled.transpose(0, 3, 1, 2)).reshape(T_SEQ, N_EMBD).astype(
            np.float32, copy=False)


def _device_forward(xc, A_list, B_list):
    """y[b] = relu(xc[b] @ A_list[b]^T) @ B_list[b] for 8 cores at once."""
    xts = _tile_X(xc)
    in_maps = []
    for b in range(NCORES):
        in_maps.append({"xt": xts[b], "at": A_list[b], "bt": B_list[b]})
    try:
        results = _get_runner()(in_maps)
    except Exception:
        # fall back to the supported dispatch path (fresh jit per call)
        from concourse.bass_utils import run_bass_kernel_spmd
        if "nc" not in _STATE:
            _STATE["nc"] = _build_nc()
        results = run_bass_kernel_spmd(
            _STATE["nc"], in_maps, list(range(NCORES))).results
    return np.stack([_untile_Y(results[b]["y"]) for b in range(NCORES)])


def _topk_mask(ck, k):
    # matches jax.lax.top_k tie-breaking (first index wins) via stable argsort
    order = np.argsort(-ck, axis=1, kind="stable")[:, :k]
    mask = np.zeros_like(ck)
    np.put_along_axis(mask, order, 1.0, axis=1)
    return mask


def kernel(x, Wm, Wp, Wc1, Wc2):
    x = np.ascontiguousarray(np.asarray(x, dtype=np.float32))
    Wm = np.ascontiguousarray(np.asarray(Wm, dtype=np.float32))
    Wp = np.ascontiguousarray(np.asarray(Wp, dtype=np.float32))
    Wc1 = np.asarray(Wc1, dtype=np.float32)
    Wc2 = np.asarray(Wc2, dtype=np.float32)
    B = x.shape[0]
    assert B == NCORES and x.shape[1] == T_SEQ and x.shape[2] == N_EMBD

    WpT = None  # lazily built; only needed on non-base iterations
    base = np.zeros((B, FULL), np.float32)
    base[:, :ITER] = 1.0

    xc, active, history, done = x, base, base.copy(), False
    for _ in range(3):
        if done:
            break
        idxs = [np.flatnonzero(active[b]) for b in range(B)]
        is_base = all(ix.shape[0] == ITER and ix[0] == 0 and ix[-1] == ITER - 1
                      for ix in idxs) and all(
                          np.array_equal(ix, idxs[0]) for ix in idxs[1:])
        if is_base and np.array_equal(idxs[0], np.arange(ITER)):
            at = _tile_A(Wm[:ITER])
            bt = _tile_B(np.ascontiguousarray(Wp[:, :ITER].T))
            A_list = [at] * B
            B_list = [bt] * B
        else:
            if WpT is None:
                WpT = np.ascontiguousarray(Wp.T)
            A_list = [_tile_A(np.ascontiguousarray(Wm[ix])) for ix in idxs]
            B_list = [_tile_B(WpT[ix]) for ix in idxs]

        y = _device_forward(xc, A_list, B_list)

        pooled = y.mean(axis=1)
        ck = np.maximum(pooled @ Wc1.T, 0.0) @ Wc2.T
        new_mask = _topk_mask(ck, ITER)
        combined = np.clip(history + new_mask, 0.0, 1.0)
        novelty = (combined - history).sum(axis=1).mean() / ITER
        xc, active, history = y, new_mask, combined
        done = bool(novelty < R_NOVELTY)

    return xc.astype(np.float32, copy=False)
